# revision 15
# baseline (speedup 1.0000x reference)
"""Trainium2 Bass kernel for nn_CBMPredictor (CGConv GNN + TDA projector + CPPN head).

Self-contained: builds, compiles and runs an 8-core SPMD Bass kernel.

Sharding: nodes are split into 8 contiguous shards (12672 per core incl.
padding); edges are sharded by destination node.  Per layer, each core:
  - builds the full "source" table B = h @ Wsrc (f||s halves) in local DRAM,
  - streams its destination blocks densely and expands them onto edges with a
    per-tile selection matmul (S matrices built on-device from dst indices),
  - gathers B[src] rows with indirect DMA,
  - computes msg = sigmoid(F) * softplus(S) and aggregates per dst block with
    a selection matmul into PSUM,
  - updates its node shard (residual + layernorm) and all-gathers the shard
    (feature-transposed) so every core has the full h for the next layer.
The pooled [128 feat, 256 graph] partials and per-graph counts are
all-reduced, and the small fusion/CPPN head is computed redundantly on every
core.
"""

import math

import numpy as np


# ----------------------------------------------------------------------------
# configuration
# ----------------------------------------------------------------------------

def full_cfg():
    return dict(
        N_REAL=100000,
        E=1600000,
        G=256,
        NC=8,
        PC=12672,          # nodes per core (99 * 128)
        H=128,
        ED=4,
        L=4,
        TDA_DIM=32,
        TDA_PROJ=64,
        K4=4,
        EPS=1e-5,
        DT="float32",      # table / message dtype: "float32" or "bfloat16"
    )


def _derived(cfg):
    cfg = dict(cfg)
    cfg["NB"] = cfg["PC"] // 128
    cfg["N_PAD"] = cfg["NC"] * cfg["PC"]
    cfg["GT"] = cfg["G"] // 128
    cfg["F_DIM"] = cfg["H"] + cfg["TDA_PROJ"]
    assert cfg["PC"] % 128 == 0 and cfg["G"] % 128 == 0
    assert cfg["NC"] * cfg["PC"] >= cfg["N_REAL"]
    return cfg


# ----------------------------------------------------------------------------
# host-side preprocessing
# ----------------------------------------------------------------------------

def _prep(cfg, inputs):
    import ml_dtypes

    NC, PC, NB = cfg["NC"], cfg["PC"], cfg["NB"]
    H, ED, L, G = cfg["H"], cfg["ED"], cfg["L"], cfg["G"]
    N_REAL, E, N_PAD = cfg["N_REAL"], cfg["E"], cfg["N_PAD"]
    npdt = np.float32 if cfg["DT"] == "float32" else ml_dtypes.bfloat16

    f32 = np.float32
    x = np.nan_to_num(np.asarray(inputs["x"], f32), nan=0.0, posinf=3.0, neginf=-3.0)
    ea = np.nan_to_num(np.asarray(inputs["edge_attr"], f32), nan=0.0, posinf=1.0,
                       neginf=0.0)
    tda = np.nan_to_num(np.asarray(inputs["tda"], f32), nan=0.0, posinf=3.0,
                        neginf=-3.0)
    ei = np.asarray(inputs["edge_index"]).astype(np.int64)
    batch_ids = np.asarray(inputs["batch_ids"]).astype(np.int64)

    src, dst = ei[0], ei[1]
    core = dst // PC
    dst_loc = dst - core * PC
    blk = dst_loc // 128
    rel = (dst_loc % 128).astype(f32)
    key = core * NB + blk
    order = np.argsort(key, kind="stable")
    cnt = np.bincount(key, minlength=NC * NB)
    B_T = max(1, int(math.ceil(cnt.max() / 128)))
    T = NB * B_T

    starts = np.concatenate([[0], np.cumsum(cnt)])
    ks = key[order]
    pos = np.arange(E) - starts[ks]
    cs = core[order]
    tile_in_blk = pos // 128
    p = pos % 128
    t_glob = blk[order] * B_T + tile_in_blk

    src_idxT = np.zeros((NC, 128, T), np.int32)
    dst_relT = np.full((NC, 128, T), -1.0, f32)
    ea1T = np.zeros((NC, 5, T * 128), f32)
    src_idxT[cs, p, t_glob] = src[order].astype(np.int32)
    dst_relT[cs, p, t_glob] = rel[order]
    slot = t_glob * 128 + p
    for d in range(ED):
        ea1T[cs, d, slot] = ea[order, d]
    ea1T[cs, 4, slot] = 1.0

    # per-core node-level arrays
    b_pad = np.full(N_PAD, -1.0, f32)
    b_pad[:N_REAL] = batch_ids.astype(f32)
    batchT = b_pad.reshape(NC, NB, 128).transpose(0, 2, 1).copy()

    x_pad = np.zeros((N_PAD, 7), f32)
    x_pad[:N_REAL] = x
    ones_pad = np.zeros(N_PAD, f32)
    ones_pad[:N_REAL] = 1.0
    xT1 = np.concatenate([x_pad, ones_pad[:, None]], axis=1)  # [N_PAD, 8]
    xT1_shard = xT1.reshape(NC, PC, 8).transpose(0, 2, 1).copy()

    # ---- weights (shared by all cores) ----
    W = {k: np.asarray(v, f32) for k, v in inputs.items()
         if k not in ("x", "edge_attr", "tda", "edge_index", "batch_ids")}

    shared = {}

    def put(name, arr, dt=None):
        shared[name] = np.ascontiguousarray(np.asarray(arr, dt or npdt))

    put("Win1", np.concatenate([W["Win"], W["b_in"][None, :]], axis=0))
    for l in range(L):
        Wf, Ws = W["Wf"][l], W["Ws"][l]
        put(f"Wdst{l}", np.concatenate([Wf[0:H], Ws[0:H]], axis=1))
        put(f"Wsrc{l}", np.concatenate([Wf[H:2 * H], Ws[H:2 * H]], axis=1))
        put(f"Wfse1{l}", np.concatenate(
            [np.concatenate([Wf[2 * H:], W["bf"][l][None, :]], axis=0),
             np.concatenate([Ws[2 * H:], W["bs"][l][None, :]], axis=0)], axis=1))
        put(f"lng{l}", np.tile(W["lng"][l][None, :], (128, 1)), f32)
        put(f"lnb{l}", np.tile(W["lnb"][l][None, :], (128, 1)), f32)

    put("identity_dt", np.eye(128, dtype=f32))
    put("identity_f32", np.eye(128, dtype=f32), f32)
    put("iota128", np.tile(np.arange(128, dtype=f32)[None, :], (128, 1)), f32)
    put("iota256", np.tile(np.arange(G, dtype=f32)[None, :], (128, 1)), f32)
    put("ones_dt", np.ones((128, 1)))
    put("eps_col", np.full((128, 1), cfg["EPS"]), f32)
    put("ones_row", np.ones((1, 128)), f32)

    # head (all f32)
    tda_pad = np.zeros((G, cfg["TDA_DIM"]), f32)
    tda_pad[:tda.shape[0]] = tda
    put("tdaT1", np.concatenate([tda_pad.T, np.ones((1, G), f32)], axis=0), f32)
    put("Wt11", np.concatenate([W["Wt1"], W["bt1"][None, :]], axis=0), f32)
    put("tg", np.tile(W["tg"][None, :], (128, 1)), f32)
    put("tb", np.tile(W["tb"][None, :], (128, 1)), f32)
    put("Wt2", W["Wt2"], f32)
    put("bt2", np.tile(W["bt2"][None, :], (128, 1)), f32)
    put("fng", np.tile(W["fng"][None, :], (128, 1)), f32)
    put("fnb", np.tile(W["fnb"][None, :], (128, 1)), f32)
    put("Wout", W["Wout"], f32)
    put("bout", np.tile(W["bout"][None, :], (128, 1)), f32)
    put("Wg1", W["Wg1"], f32)
    put("bg1_row", W["bg1"][None, :], f32)
    put("Wg2", W["Wg2"], f32)
    put("bg2_row", W["bg2"][None, :], f32)
    put("WplT", W["Wpl"].T, f32)
    # fold bq2 into the lin bias (preds = lin + quad, both biased)
    put("bplq_row", (W["bpl"] + W["bq2"])[None, :], f32)
    K4, F_DIM = cfg["K4"], cfg["F_DIM"]
    FH = F_DIM // 2
    put("Wq1f", W["Wq1"].transpose(1, 0, 2).reshape(F_DIM, K4 * FH), f32)
    put("bq1_row", W["bq1"].reshape(1, K4 * FH), f32)
    put("Wq2bc", np.tile(W["Wq2"].reshape(1, K4 * FH), (128, 1)), f32)

    in_maps = []
    for c in range(NC):
        m = dict(shared)
        m["src_idxT"] = np.ascontiguousarray(src_idxT[c])
        m["dst_relT"] = np.ascontiguousarray(dst_relT[c])
        m["ea1T"] = np.ascontiguousarray(ea1T[c].astype(npdt))
        m["batchT"] = np.ascontiguousarray(batchT[c])
        m["xT1_shard"] = np.ascontiguousarray(xT1_shard[c].astype(npdt))
        in_maps.append(m)
    return in_maps, B_T


# ----------------------------------------------------------------------------
# kernel build
# ----------------------------------------------------------------------------

def _build(cfg, B_T):
    import concourse.bacc as bacc
    import concourse.bass as bass
    import concourse.mybir as mybir
    from concourse.tile import TileContext

    NC, PC, NB = cfg["NC"], cfg["PC"], cfg["NB"]
    H, L, G, GT = cfg["H"], cfg["L"], cfg["G"], cfg["GT"]
    N_PAD, EPS = cfg["N_PAD"], cfg["EPS"]
    K4, F_DIM = cfg["K4"], cfg["F_DIM"]
    FH = F_DIM // 2
    T = NB * B_T
    DT = mybir.dt.float32 if cfg["DT"] == "float32" else mybir.dt.bfloat16
    F32 = mybir.dt.float32
    AF = mybir.ActivationFunctionType
    OP = mybir.AluOpType
    ds, ts = bass.ds, bass.ts

    nc = bacc.Bacc("TRN2")

    # ---- I/O ----
    inp = {}

    def di(name, shape, dt=DT):
        inp[name] = nc.dram_tensor(name, shape, dt, kind="ExternalInput")
        return inp[name]

    src_idxT = di("src_idxT", [128, T], mybir.dt.int32)
    dst_relT = di("dst_relT", [128, T], F32)
    ea1T = di("ea1T", [5, T * 128])
    batchT = di("batchT", [128, NB], F32)
    xT1_shard = di("xT1_shard", [8, PC])
    Win1 = di("Win1", [8, H])
    Wdst = [di(f"Wdst{l}", [H, 2 * H]) for l in range(L)]
    Wsrc = [di(f"Wsrc{l}", [H, 2 * H]) for l in range(L)]
    Wfse1 = [di(f"Wfse1{l}", [5, 2 * H]) for l in range(L)]
    lng = [di(f"lng{l}", [128, H], F32) for l in range(L)]
    lnb = [di(f"lnb{l}", [128, H], F32) for l in range(L)]
    identity_dt = di("identity_dt", [128, 128])
    identity_f32 = di("identity_f32", [128, 128], F32)
    iota128 = di("iota128", [128, 128], F32)
    iota256 = di("iota256", [128, G], F32)
    ones_dt = di("ones_dt", [128, 1])
    eps_col = di("eps_col", [128, 1], F32)
    ones_row = di("ones_row", [1, 128], F32)
    tdaT1 = di("tdaT1", [cfg["TDA_DIM"] + 1, G], F32)
    Wt11 = di("Wt11", [cfg["TDA_DIM"] + 1, 2 * cfg["TDA_PROJ"]], F32)
    tg = di("tg", [128, 2 * cfg["TDA_PROJ"]], F32)
    tb = di("tb", [128, 2 * cfg["TDA_PROJ"]], F32)
    Wt2 = di("Wt2", [2 * cfg["TDA_PROJ"], cfg["TDA_PROJ"]], F32)
    bt2 = di("bt2", [128, cfg["TDA_PROJ"]], F32)
    fng = di("fng", [128, F_DIM], F32)
    fnb = di("fnb", [128, F_DIM], F32)
    Wout = di("Wout", [H, H], F32)
    bout = di("bout", [128, H], F32)
    Wg1 = di("Wg1", [F_DIM, K4 * 4], F32)
    bg1_row = di("bg1_row", [1, K4 * 4], F32)
    Wg2 = di("Wg2", [K4 * 4, K4], F32)
    bg2_row = di("bg2_row", [1, K4], F32)
    WplT = di("WplT", [F_DIM, K4], F32)
    bplq_row = di("bplq_row", [1, K4], F32)
    Wq1f = di("Wq1f", [F_DIM, K4 * FH], F32)
    bq1_row = di("bq1_row", [1, K4 * FH], F32)
    Wq2bc = di("Wq2bc", [128, K4 * FH], F32)

    y_out = nc.dram_tensor("y", [G], F32, kind="ExternalOutput")
    z_out = nc.dram_tensor("z", [G, F_DIM], F32, kind="ExternalOutput")
    DBG = cfg.get("DEBUG", False)
    if DBG:
        dbg_h = [nc.dram_tensor(f"dbg_h{i}", [128, PC], F32,
                                kind="ExternalOutput") for i in range(L + 1)]
        dbg_agg = [nc.dram_tensor(f"dbg_agg{i}", [128, PC], F32,
                                  kind="ExternalOutput") for i in range(L)]
        dbg_pool = nc.dram_tensor("dbg_pool", [128, G + 2], F32,
                                  kind="ExternalOutput")

    # ---- internal DRAM ----
    h_shardT_d = nc.dram_tensor("h_shardT_d", [128, PC], DT)
    h_fullT = nc.dram_tensor("h_fullT", [NC * 128, PC], DT, addr_space="Shared")
    B_table = nc.dram_tensor("B_table", [N_PAD, 2 * H], DT)
    pool_in = nc.dram_tensor("pool_in", [128, G + 2], F32)
    pool_out = nc.dram_tensor("pool_out", [128, G + 2], F32, addr_space="Shared")

    groups = [list(range(NC))]

    with nc.allow_low_precision(reason="bf16 transposes/copies by design"), \
         TileContext(nc) as tc:
        with (
            tc.tile_pool(name="const", bufs=1) as cp,
            tc.tile_pool(name="res", bufs=1) as rp,
            tc.tile_pool(name="blk", bufs=2) as bp,
            tc.tile_pool(name="tile", bufs=3) as tp,
            tc.tile_pool(name="node", bufs=2) as np_,
            tc.tile_pool(name="bb", bufs=2) as bbp,
            tc.tile_pool(name="ps_t", bufs=2, space="PSUM") as ps_t,
            tc.tile_pool(name="ps_f", bufs=2, space="PSUM") as ps_f,
            tc.tile_pool(name="ps_a", bufs=2, space="PSUM") as ps_a,
            tc.tile_pool(name="ps_b", bufs=1, space="PSUM") as ps_b,
        ):
            # ---- load constants ----
            def load_const(t, dt=DT):
                tile = cp.tile(list(t.shape), dt, tag=t.name)
                nc.sync.dma_start(out=tile[:], in_=t[:])
                return tile

            c_id = load_const(identity_dt)
            c_idf = load_const(identity_f32, F32)
            c_iota = load_const(iota128, F32)
            c_iota256 = load_const(iota256, F32)
            c_ones = load_const(ones_dt)
            c_eps = load_const(eps_col, F32)
            c_ones_row = load_const(ones_row, F32)
            c_Win1 = load_const(Win1)
            c_Wdst = [load_const(Wdst[l]) for l in range(L)]
            c_Wsrc = [load_const(Wsrc[l]) for l in range(L)]
            c_Wfse1 = [load_const(Wfse1[l]) for l in range(L)]
            c_lng = [load_const(lng[l], F32) for l in range(L)]
            c_lnb = [load_const(lnb[l], F32) for l in range(L)]
            c_batchT = load_const(batchT, F32)

            hsh = rp.tile([128, PC], DT)
            hshT = rp.tile([128, PC], DT)
            agg = rp.tile([128, PC], F32)

            # ---------------- layer-norm helper (in/out [128, W] f32) -------
            def layer_norm(dst_ap, src_ap, w, g_tile, b_tile, pool):
                nm = pool.tile([128, 1], F32, tag="ln_nm")
                nm2 = pool.tile([128, 1], F32, tag="ln_nm2")
                xc = pool.tile([128, w], F32, tag="ln_xc")
                sq = pool.tile([128, w], F32, tag="ln_sq")
                vs = pool.tile([128, 1], F32, tag="ln_vs")
                rs = pool.tile([128, 1], F32, tag="ln_rs")
                nc.vector.tensor_reduce(out=nm[:], in_=src_ap,
                                        axis=mybir.AxisListType.X, op=OP.add,
                                        negate=True)
                nc.scalar.activation(nm2[:], nm[:], AF.Copy, scale=1.0 / w)
                nc.vector.tensor_scalar_add(out=xc[:], in0=src_ap, scalar1=nm2[:])
                nc.scalar.activation(sq[:], xc[:], AF.Square, accum_out=vs[:])
                sv = pool.tile([128, 1], F32, tag="ln_sv")
                nc.scalar.activation(sv[:], vs[:], AF.Sqrt, scale=1.0 / w,
                                     bias=c_eps[:])
                nc.vector.reciprocal(rs[:], sv[:])
                tmp = pool.tile([128, w], F32, tag="ln_tmp")
                nc.scalar.activation(tmp[:], xc[:], AF.Copy, scale=rs[:])
                tmp2 = pool.tile([128, w], F32, tag="ln_tmp2")
                nc.vector.tensor_tensor(out=tmp2[:], in0=tmp[:], in1=g_tile,
                                        op=OP.mult)
                nc.vector.tensor_tensor(out=dst_ap, in0=tmp2[:], in1=b_tile,
                                        op=OP.add)

            # ---------------- phase 0: h0 ----------------
            def h0_body(tb):
                xt = np_.tile([8, 128], DT, tag="h0_x")
                nc.sync.dma_start(out=xt[:], in_=xT1_shard[:, ts(tb, 128)])
                pm = ps_t.tile([128, 128], F32, tag="pt128")
                nc.tensor.matmul(pm[:], xt[:], c_Win1[:], start=True, stop=True)
                h0t = np_.tile([128, 128], DT, tag="h0_t")
                nc.scalar.activation(h0t[:], pm[:], AF.Silu)
                nc.vector.tensor_copy(out=hsh[:, ts(tb, 128)], in_=h0t[:])
                pt = ps_t.tile([128, 128], DT, tag="pt128")
                nc.tensor.transpose(pt[:], h0t[:], c_id[:])
                nc.scalar.copy(out=hshT[:, ts(tb, 128)], in_=pt[:])

            tc.For_i_unrolled(0, NB, 1, h0_body, max_unroll=min(8, NB))
            if DBG:
                nc.sync.dma_start(out=dbg_h[0][:], in_=hsh[:])
            nc.sync.dma_start(out=h_shardT_d[:], in_=hshT[:])
            nc.gpsimd.collective_compute(
                "AllGather", OP.bypass, ins=[h_shardT_d[:]], outs=[h_fullT[:]],
                replica_groups=groups)

            # ---------------- layers ----------------
            for l in range(L):
                # -- build B table from h_fullT --
                def bb_body(g, s):
                    hT = bbp.tile([128, 384], DT, tag="bb_h")
                    nc.sync.dma_start(
                        out=hT[:],
                        in_=h_fullT[s * 128:(s + 1) * 128, ds(g * 384, 384)])
                    pm = ps_b.tile([128, 768], F32, tag="pb768")
                    for i in range(3):
                        nc.tensor.matmul(pm[:, i * 256:(i + 1) * 256],
                                         hT[:, i * 128:(i + 1) * 128],
                                         c_Wsrc[l][:], start=True, stop=True)
                    bt = bbp.tile([128, 768], DT, tag="bb_bt")
                    nc.scalar.copy(out=bt[:], in_=pm[:])
                    out_ap = B_table[ds(s * PC + g * 384, 384), :].rearrange(
                        "(i p) d -> p i d", p=128)
                    nc.sync.dma_start(out=out_ap,
                                      in_=bt[:].rearrange("p (i d) -> p i d", i=3))

                for s in range(NC):
                    tc.For_i_unrolled(0, NB // 3, 1,
                                      lambda g, s=s: bb_body(g, s),
                                      max_unroll=min(11, NB // 3))

                # -- edge phase --
                def edge_body(b):
                    idx_sb = bp.tile([128, B_T], mybir.dt.int32, tag="e_idx")
                    nc.sync.dma_start(out=idx_sb[:], in_=src_idxT[:, ts(b, B_T)])
                    rel_sb = bp.tile([128, B_T], F32, tag="e_rel")
                    nc.sync.dma_start(out=rel_sb[:], in_=dst_relT[:, ts(b, B_T)])
                    ea_sb = bp.tile([5, B_T * 128], DT, tag="e_ea")
                    nc.sync.dma_start(out=ea_sb[:], in_=ea1T[:, ts(b, B_T * 128)])
                    blkT = bp.tile([128, 128], DT, tag="e_hT")
                    nc.vector.tensor_copy(out=blkT[:], in_=hshT[:, ts(b, 128)])
                    pa = ps_b.tile([128, 256], F32, tag="pb768")
                    nc.tensor.matmul(pa[:], blkT[:], c_Wdst[l][:],
                                     start=True, stop=True)
                    A_sb = bp.tile([128, 256], DT, tag="e_A")
                    nc.scalar.copy(out=A_sb[:], in_=pa[:])
                    S_cat = bp.tile([128, B_T * 128], DT, tag="e_S")
                    nc.vector.tensor_tensor(
                        out=S_cat[:].rearrange("p (t w) -> p t w", t=B_T),
                        in0=rel_sb[:].unsqueeze(2).to_broadcast([128, B_T, 128]),
                        in1=c_iota[:].unsqueeze(1).to_broadcast([128, B_T, 128]),
                        op=OP.is_equal)
                    pagg = ps_a.tile([128, 128], F32, tag="pa128")
                    for i in range(B_T):
                        st_p = ps_t.tile([128, 128], DT, tag="pt128")
                        nc.tensor.transpose(st_p[:],
                                            S_cat[:, i * 128:(i + 1) * 128],
                                            c_id[:])
                        ST = tp.tile([128, 128], DT, tag="e_ST")
                        nc.scalar.copy(out=ST[:], in_=st_p[:])
                        pf = ps_f.tile([128, 256], F32, tag="pf")
                        nc.tensor.matmul(pf[:], ST[:], A_sb[:], start=True,
                                         stop=False)
                        nc.tensor.matmul(pf[:], ea_sb[:, i * 128:(i + 1) * 128],
                                         c_Wfse1[l][:], start=False, stop=True)
                        Gt = tp.tile([128, 256], DT, tag="e_G")
                        nc.gpsimd.indirect_dma_start(
                            out=Gt[:], out_offset=None, in_=B_table[:],
                            in_offset=bass.IndirectOffsetOnAxis(
                                ap=idx_sb[:, i:i + 1], axis=0))
                        Ft = tp.tile([128, 256], DT, tag="e_F")
                        nc.vector.tensor_tensor(out=Ft[:], in0=Gt[:], in1=pf[:],
                                                op=OP.add)
                        # sigmoid(f)*softplus(s) via Exp/Ln (same ACT table)
                        ef = tp.tile([128, 128], DT, tag="e_ef")
                        nc.scalar.activation(ef[:], Ft[:, 0:128], AF.Exp,
                                             scale=-1.0)
                        es = tp.tile([128, 128], DT, tag="e_es")
                        nc.scalar.activation(es[:], Ft[:, 128:256], AF.Exp)
                        t1 = tp.tile([128, 128], DT, tag="e_t1")
                        nc.vector.tensor_scalar_add(out=t1[:], in0=ef[:],
                                                    scalar1=1.0)
                        rc = tp.tile([128, 128], DT, tag="e_rc")
                        nc.vector.reciprocal(rc[:], t1[:])
                        t3 = tp.tile([128, 128], DT, tag="e_t3")
                        nc.vector.tensor_scalar_add(out=t3[:], in0=es[:],
                                                    scalar1=1.0)
                        lv = tp.tile([128, 128], DT, tag="e_lv")
                        nc.scalar.activation(lv[:], t3[:], AF.Ln)
                        msg = tp.tile([128, 128], DT, tag="e_msg")
                        nc.vector.tensor_tensor(out=msg[:], in0=rc[:], in1=lv[:],
                                                op=OP.mult)
                        nc.tensor.matmul(pagg[:],
                                         S_cat[:, i * 128:(i + 1) * 128],
                                         msg[:], start=(i == 0),
                                         stop=(i == B_T - 1))
                    nc.scalar.copy(out=agg[:, ts(b, 128)], in_=pagg[:])

                tc.For_i_unrolled(0, NB, 1, edge_body, max_unroll=min(2, NB))
                if DBG:
                    nc.sync.dma_start(out=dbg_agg[l][:], in_=agg[:])

                # -- node update --
                def node_body(tb):
                    sl = ts(tb, 128)
                    conv = np_.tile([128, 128], F32, tag="n_conv")
                    nc.vector.tensor_tensor(out=conv[:], in0=agg[:, sl],
                                            in1=hsh[:, sl], op=OP.add)
                    t2a = np_.tile([128, 128], F32, tag="n_t2a")
                    nc.scalar.activation(t2a[:], conv[:], AF.Silu)
                    t2 = np_.tile([128, 128], F32, tag="n_t2")
                    nc.vector.tensor_tensor(out=t2[:], in0=t2a[:], in1=hsh[:, sl],
                                            op=OP.add)
                    nh = np_.tile([128, 128], DT, tag="n_nh")
                    layer_norm(nh[:], t2[:], 128, c_lng[l][:], c_lnb[l][:],
                               np_)
                    nc.vector.tensor_copy(out=hsh[:, sl], in_=nh[:])
                    pt = ps_t.tile([128, 128], DT, tag="pt128")
                    nc.tensor.transpose(pt[:], nh[:], c_id[:])
                    nc.scalar.copy(out=hshT[:, sl], in_=pt[:])

                tc.For_i_unrolled(0, NB, 1, node_body, max_unroll=min(4, NB))
                if DBG:
                    nc.sync.dma_start(out=dbg_h[l + 1][:], in_=hsh[:])

                if l < L - 1:
                    nc.sync.dma_start(out=h_shardT_d[:], in_=hshT[:])
                    nc.gpsimd.collective_compute(
                        "AllGather", OP.bypass, ins=[h_shardT_d[:]],
                        outs=[h_fullT[:]], replica_groups=groups)

            # ---------------- pooling ----------------
            p_pool = ps_b.tile([128, G], F32, tag="pb768")
            p_cnts = [ps_a.tile([128, 1], F32, tag="pa128", name=f"p_cnt{_i}")
                      for _i in range(GT)]

            def pool_body(tb, first, last):
                sp_t = np_.tile([128, G], DT, tag="p_S")
                nc.vector.tensor_tensor(
                    out=sp_t[:],
                    in0=c_batchT[:, ds(tb, 1)].to_broadcast([128, G]),
                    in1=c_iota256[:], op=OP.is_equal)
                hcp = np_.tile([128, 128], DT, tag="p_h")
                nc.vector.tensor_copy(out=hcp[:], in_=hsh[:, ts(tb, 128)])
                nc.tensor.matmul(p_pool[:], hcp[:], sp_t[:],
                                 start=first, stop=last)
                for hh in range(GT):
                    nc.tensor.matmul(p_cnts[hh][:],
                                     sp_t[:, hh * 128:(hh + 1) * 128],
                                     c_ones[:], start=first, stop=last)

            pool_body(0, True, False)
            if NB > 2:
                tc.For_i_unrolled(1, NB - 1, 1,
                                  lambda tb: pool_body(tb, False, False),
                                  max_unroll=min(8, NB - 2))
            pool_body(NB - 1, False, True)

            pool_sb = np_.tile([128, G + 2], F32, tag="p_sb")
            nc.scalar.copy(out=pool_sb[:, 0:G], in_=p_pool[:])
            for hh in range(GT):
                nc.scalar.copy(out=pool_sb[:, G + hh:G + hh + 1], in_=p_cnts[hh][:])
            nc.sync.dma_start(out=pool_in[:], in_=pool_sb[:])
            nc.gpsimd.collective_compute(
                "AllReduce", OP.add, ins=[pool_in[:]], outs=[pool_out[:]],
                replica_groups=groups)
            pl = np_.tile([128, G + 2], F32, tag="p_pl")
            nc.sync.dma_start(out=pl[:], in_=pool_out[:])
            if DBG:
                nc.sync.dma_start(out=dbg_pool[:], in_=pool_out[:])

            # ---------------- head ----------------
            hp = np_  # pool for head tiles
            c_tdaT1 = load_const(tdaT1, F32)
            c_Wt11 = load_const(Wt11, F32)
            c_tg = load_const(tg, F32)
            c_tb = load_const(tb, F32)
            c_Wt2 = load_const(Wt2, F32)
            c_bt2 = load_const(bt2, F32)
            c_fng = load_const(fng, F32)
            c_fnb = load_const(fnb, F32)
            c_Wout = load_const(Wout, F32)
            c_bout = load_const(bout, F32)
            def load_const_pair(t):
                a = cp.tile([128, t.shape[1]], F32, tag=t.name + "_a")
                nc.sync.dma_start(out=a[:], in_=t[0:128, :])
                b = cp.tile([t.shape[0] - 128, t.shape[1]], F32, tag=t.name + "_b")
                nc.sync.dma_start(out=b[:], in_=t[128:, :])
                return a, b

            c_Wg1a, c_Wg1b = load_const_pair(Wg1)
            c_bg1r = load_const(bg1_row, F32)
            c_Wg2 = load_const(Wg2, F32)
            c_bg2r = load_const(bg2_row, F32)
            c_WplTa, c_WplTb = load_const_pair(WplT)
            c_bplqr = load_const(bplq_row, F32)
            c_Wq1fa, c_Wq1fb = load_const_pair(Wq1f)
            c_bq1r = load_const(bq1_row, F32)
            c_Wq2bc = load_const(Wq2bc, F32)

            # counts reciprocal per half: pl[:, G+hh]
            rcp = []
            for hh in range(GT):
                cc = hp.tile([128, 1], F32, tag=f"hd_c{hh}")
                nc.vector.tensor_scalar_max(out=cc[:], in0=pl[:, G + hh:G + hh + 1],
                                            scalar1=1.0)
                rr = hp.tile([128, 1], F32, tag=f"hd_r{hh}")
                nc.vector.reciprocal(rr[:], cc[:])
                rcp.append(rr)

            zT0 = hp.tile([128, G], F32, tag="hd_zT0")
            zT1 = hp.tile([64, G], F32, tag="hd_zT1")
            for hh in range(GT):
                gsl = slice(hh * 128, (hh + 1) * 128)
                # z_gnn = (pooledT.T @ Wout) * (1/cnt) + bout
                pz = ps_f.tile([128, 128], F32, tag="pf")
                nc.tensor.matmul(pz[:], pl[:, gsl], c_Wout[:], start=True,
                                 stop=True)
                zg = hp.tile([128, F_DIM], F32, tag="hd_z")
                nc.scalar.activation(zg[:, 0:H], pz[:], AF.Copy, scale=rcp[hh][:])
                nc.vector.tensor_tensor(out=zg[:, 0:H], in0=zg[:, 0:H],
                                        in1=c_bout[:], op=OP.add)
                # tda projector
                pt1 = ps_f.tile([128, 2 * cfg["TDA_PROJ"]], F32, tag="pf")
                nc.tensor.matmul(pt1[:], c_tdaT1[:, gsl], c_Wt11[:], start=True,
                                 stop=True)
                t1 = hp.tile([128, 2 * cfg["TDA_PROJ"]], F32, tag="hd_t1")
                nc.scalar.activation(t1[:], pt1[:], AF.Silu)
                t1n = hp.tile([128, 2 * cfg["TDA_PROJ"]], F32, tag="hd_t1n")
                layer_norm(t1n[:], t1[:], 2 * cfg["TDA_PROJ"], c_tg[:], c_tb[:],
                           hp)
                ptt = ps_t.tile([128, 128], F32, tag="pt128")
                nc.tensor.transpose(ptt[:], t1n[:], c_idf[:])
                t1T = hp.tile([128, 128], F32, tag="hd_t1T")
                nc.scalar.copy(out=t1T[:], in_=ptt[:])
                pzt = ps_f.tile([128, cfg["TDA_PROJ"]], F32, tag="pf")
                nc.tensor.matmul(pzt[:], t1T[:], c_Wt2[:], start=True, stop=True)
                nc.vector.tensor_tensor(out=zg[:, H:F_DIM], in0=pzt[:],
                                        in1=c_bt2[:], op=OP.add)
                # fusion layernorm
                zf = hp.tile([128, F_DIM], F32, tag="hd_zf")
                layer_norm(zf[:], zg[:], F_DIM, c_fng[:], c_fnb[:], hp)
                nc.sync.dma_start(out=z_out[gsl, :], in_=zf[:])
                # transposes into zT0/zT1
                pz0 = ps_t.tile([128, 128], F32, tag="pt128")
                nc.tensor.transpose(pz0[:], zf[:, 0:128], c_idf[:])
                nc.scalar.copy(out=zT0[:, gsl], in_=pz0[:])
                pz1 = ps_t.tile([64, 128], F32, tag="pt128")
                nc.tensor.transpose(pz1[:], zf[:, 128:192], c_idf[:])
                nc.scalar.copy(out=zT1[:, gsl], in_=pz1[:])

            for hh in range(GT):
                gsl = slice(hh * 128, (hh + 1) * 128)
                # gates
                pg1 = ps_f.tile([128, K4 * 4], F32, tag="pf")
                nc.tensor.matmul(pg1[:], zT0[:, gsl], c_Wg1a[:],
                                 start=True, stop=False)
                nc.tensor.matmul(pg1[:], zT1[:, gsl], c_Wg1b[:],
                                 start=False, stop=False)
                nc.tensor.matmul(pg1[:], c_ones_row[:], c_bg1r[:],
                                 start=False, stop=True)
                g1 = hp.tile([128, K4 * 4], F32, tag="hd_g1")
                nc.scalar.activation(g1[:], pg1[:], AF.Silu)
                pg1t = ps_t.tile([16, 128], F32, tag="pt128")
                nc.tensor.transpose(pg1t[:], g1[:], c_idf[:])
                g1T = hp.tile([16, 128], F32, tag="hd_g1T")
                nc.scalar.copy(out=g1T[:], in_=pg1t[:])
                pg2 = ps_f.tile([128, K4], F32, tag="pf")
                nc.tensor.matmul(pg2[:], g1T[:], c_Wg2[:], start=True, stop=False)
                nc.tensor.matmul(pg2[:], c_ones_row[:], c_bg2r[:],
                                 start=False, stop=True)
                ex = hp.tile([128, K4], F32, tag="hd_ex")
                se = hp.tile([128, 1], F32, tag="hd_se")
                nc.scalar.activation(ex[:], pg2[:], AF.Exp, accum_out=se[:])
                rse = hp.tile([128, 1], F32, tag="hd_rse")
                nc.vector.reciprocal(rse[:], se[:])
                gates = hp.tile([128, K4], F32, tag="hd_gates")
                nc.scalar.activation(gates[:], ex[:], AF.Copy, scale=rse[:])
                # lin (+ folded bq2)
                plin = ps_f.tile([128, K4], F32, tag="pf")
                nc.tensor.matmul(plin[:], zT0[:, gsl], c_WplTa[:],
                                 start=True, stop=False)
                nc.tensor.matmul(plin[:], zT1[:, gsl], c_WplTb[:],
                                 start=False, stop=False)
                nc.tensor.matmul(plin[:], c_ones_row[:], c_bplqr[:],
                                 start=False, stop=True)
                # quad
                pq = ps_b.tile([128, K4 * FH], F32, tag="pb768")
                nc.tensor.matmul(pq[:], zT0[:, gsl], c_Wq1fa[:],
                                 start=True, stop=False)
                nc.tensor.matmul(pq[:], zT1[:, gsl], c_Wq1fb[:],
                                 start=False, stop=False)
                nc.tensor.matmul(pq[:], c_ones_row[:], c_bq1r[:],
                                 start=False, stop=True)
                qs = hp.tile([128, K4 * FH], F32, tag="hd_qs")
                nc.scalar.activation(qs[:], pq[:], AF.Silu)
                qw = hp.tile([128, K4 * FH], F32, tag="hd_qw")
                nc.vector.tensor_tensor(out=qw[:], in0=qs[:], in1=c_Wq2bc[:],
                                        op=OP.mult)
                quad = hp.tile([128, K4], F32, tag="hd_quad")
                nc.vector.tensor_reduce(
                    out=quad[:],
                    in_=qw[:].rearrange("p (k f) -> p k f", k=K4),
                    axis=mybir.AxisListType.X, op=OP.add)
                preds = hp.tile([128, K4], F32, tag="hd_preds")
                nc.vector.tensor_tensor(out=preds[:], in0=quad[:], in1=plin[:],
                                        op=OP.add)
                gp_t = hp.tile([128, K4], F32, tag="hd_gp")
                nc.vector.tensor_tensor(out=gp_t[:], in0=gates[:], in1=preds[:],
                                        op=OP.mult)
                yv = hp.tile([128, 1], F32, tag="hd_y")
                nc.vector.tensor_reduce(out=yv[:], in_=gp_t[:],
                                        axis=mybir.AxisListType.X, op=OP.add)
                nc.sync.dma_start(out=y_out[gsl].unsqueeze(1), in_=yv[:])

    nc.compile()
    return nc


# ----------------------------------------------------------------------------
# entry point
# ----------------------------------------------------------------------------

def run(cfg, inputs, trace=False):
    from concourse.bass_utils import run_bass_kernel_spmd

    cfg = _derived(cfg)
    in_maps, B_T = _prep(cfg, inputs)
    nc = _build(cfg, B_T)
    res = run_bass_kernel_spmd(nc, in_maps, list(range(cfg["NC"])), trace=trace)
    y = res.results[0]["y"][:cfg["G"]]
    z = res.results[0]["z"][:cfg["G"]]
    return (y, z), res


def kernel(**inputs):
    (y, z), _ = run(full_cfg(), inputs, trace=False)
    return y, z


# revision 18
# speedup vs baseline: 1.1203x; 1.1203x over previous
"""Trainium2 Bass kernel for nn_CBMPredictor (CGConv GNN + TDA projector + CPPN head).

Self-contained: builds, compiles and runs an 8-core SPMD Bass kernel.

Sharding: nodes are split into 8 contiguous shards (12672 per core incl.
padding); edges are sharded by destination node.  Per layer, each core:
  - builds the full "source" table B = h @ Wsrc (f||s halves) in local DRAM,
  - streams its destination blocks densely and expands them onto edges with a
    per-tile selection matmul (S matrices built on-device from dst indices),
  - gathers B[src] rows with indirect DMA,
  - computes msg = sigmoid(F) * softplus(S) and aggregates per dst block with
    a selection matmul into PSUM,
  - updates its node shard (residual + layernorm) and all-gathers the shard
    (feature-transposed) so every core has the full h for the next layer.
The pooled [128 feat, 256 graph] partials and per-graph counts are
all-reduced, and the small fusion/CPPN head is computed redundantly on every
core.
"""

import math

import numpy as np


# ----------------------------------------------------------------------------
# configuration
# ----------------------------------------------------------------------------

def full_cfg():
    return dict(
        N_REAL=100000,
        E=1600000,
        G=256,
        NC=8,
        PC=12672,          # nodes per core (99 * 128)
        H=128,
        ED=4,
        L=4,
        TDA_DIM=32,
        TDA_PROJ=64,
        K4=4,
        EPS=1e-5,
        DT="float32",      # table / message dtype: "float32" or "bfloat16"
    )


def _derived(cfg):
    cfg = dict(cfg)
    cfg["NB"] = cfg["PC"] // 128
    cfg["N_PAD"] = cfg["NC"] * cfg["PC"]
    cfg["GT"] = cfg["G"] // 128
    cfg["F_DIM"] = cfg["H"] + cfg["TDA_PROJ"]
    assert cfg["PC"] % 128 == 0 and cfg["G"] % 128 == 0
    assert cfg["NC"] * cfg["PC"] >= cfg["N_REAL"]
    return cfg


# ----------------------------------------------------------------------------
# host-side preprocessing
# ----------------------------------------------------------------------------

def _prep(cfg, inputs):
    import ml_dtypes

    NC, PC, NB = cfg["NC"], cfg["PC"], cfg["NB"]
    H, ED, L, G = cfg["H"], cfg["ED"], cfg["L"], cfg["G"]
    N_REAL, E, N_PAD = cfg["N_REAL"], cfg["E"], cfg["N_PAD"]
    npdt = np.float32 if cfg["DT"] == "float32" else ml_dtypes.bfloat16

    f32 = np.float32
    x = np.nan_to_num(np.asarray(inputs["x"], f32), nan=0.0, posinf=3.0, neginf=-3.0)
    ea = np.nan_to_num(np.asarray(inputs["edge_attr"], f32), nan=0.0, posinf=1.0,
                       neginf=0.0)
    tda = np.nan_to_num(np.asarray(inputs["tda"], f32), nan=0.0, posinf=3.0,
                        neginf=-3.0)
    ei = np.asarray(inputs["edge_index"]).astype(np.int64)
    batch_ids = np.asarray(inputs["batch_ids"]).astype(np.int64)

    src, dst = ei[0], ei[1]
    core = dst // PC
    dst_loc = dst - core * PC
    blk = dst_loc // 128
    rel = (dst_loc % 128).astype(f32)
    key = core * NB + blk
    order = np.argsort(key, kind="stable")
    cnt = np.bincount(key, minlength=NC * NB)
    B_T = max(1, int(math.ceil(cnt.max() / 128)))
    T = NB * B_T

    starts = np.concatenate([[0], np.cumsum(cnt)])
    ks = key[order]
    pos = np.arange(E) - starts[ks]
    cs = core[order]
    tile_in_blk = pos // 128
    p = pos % 128
    t_glob = blk[order] * B_T + tile_in_blk

    src_idxT = np.zeros((NC, 128, T), np.int32)
    dst_relT = np.full((NC, 128, T), -1.0, f32)
    ea1T = np.zeros((NC, 5, T * 128), f32)
    src_idxT[cs, p, t_glob] = src[order].astype(np.int32)
    dst_relT[cs, p, t_glob] = rel[order]
    slot = t_glob * 128 + p
    for d in range(ED):
        ea1T[cs, d, slot] = ea[order, d]
    ea1T[cs, 4, slot] = 1.0
    ST_host = np.zeros((NC, 128, T * 128), npdt)
    ST_host[cs, rel[order].astype(np.int64), slot] = 1.0

    # per-core node-level arrays
    b_pad = np.full(N_PAD, -1.0, f32)
    b_pad[:N_REAL] = batch_ids.astype(f32)
    batchT = b_pad.reshape(NC, NB, 128).transpose(0, 2, 1).copy()

    x_pad = np.zeros((N_PAD, 7), f32)
    x_pad[:N_REAL] = x
    ones_pad = np.zeros(N_PAD, f32)
    ones_pad[:N_REAL] = 1.0
    xT1 = np.concatenate([x_pad, ones_pad[:, None]], axis=1)  # [N_PAD, 8]
    xT1_shard = xT1.reshape(NC, PC, 8).transpose(0, 2, 1).copy()

    # ---- weights (shared by all cores) ----
    W = {k: np.asarray(v, f32) for k, v in inputs.items()
         if k not in ("x", "edge_attr", "tda", "edge_index", "batch_ids")}

    shared = {}

    def put(name, arr, dt=None):
        shared[name] = np.ascontiguousarray(np.asarray(arr, dt or npdt))

    put("Win1", np.concatenate([W["Win"], W["b_in"][None, :]], axis=0))
    for l in range(L):
        # f-gate columns are NEGATED so exp(F) directly yields e^{-f} || e^{s}
        Wf, Ws = W["Wf"][l], W["Ws"][l]
        put(f"Wdst{l}", np.concatenate([-Wf[0:H], Ws[0:H]], axis=1))
        put(f"Wsrc{l}", np.concatenate([-Wf[H:2 * H], Ws[H:2 * H]], axis=1))
        put(f"Wfse1{l}", np.concatenate(
            [np.concatenate([-Wf[2 * H:], -W["bf"][l][None, :]], axis=0),
             np.concatenate([Ws[2 * H:], W["bs"][l][None, :]], axis=0)], axis=1))
        put(f"lng{l}", np.tile(W["lng"][l][None, :], (128, 1)), f32)
        put(f"lnb{l}", np.tile(W["lnb"][l][None, :], (128, 1)), f32)

    put("identity_dt", np.eye(128, dtype=f32))
    put("identity_f32", np.eye(128, dtype=f32), f32)
    put("iota128", np.tile(np.arange(128, dtype=f32)[None, :], (128, 1)), f32)
    put("iota256", np.tile(np.arange(G, dtype=f32)[None, :], (128, 1)), f32)
    put("ones_dt", np.ones((128, 1)))
    put("eps_col", np.full((128, 1), cfg["EPS"]), f32)
    put("one_col", np.ones((128, 1)), f32)
    put("ones_row", np.ones((1, 128)), f32)

    # head (all f32)
    tda_pad = np.zeros((G, cfg["TDA_DIM"]), f32)
    tda_pad[:tda.shape[0]] = tda
    put("tdaT1", np.concatenate([tda_pad.T, np.ones((1, G), f32)], axis=0), f32)
    put("Wt11", np.concatenate([W["Wt1"], W["bt1"][None, :]], axis=0), f32)
    put("tg", np.tile(W["tg"][None, :], (128, 1)), f32)
    put("tb", np.tile(W["tb"][None, :], (128, 1)), f32)
    put("Wt2", W["Wt2"], f32)
    put("bt2", np.tile(W["bt2"][None, :], (128, 1)), f32)
    put("fng", np.tile(W["fng"][None, :], (128, 1)), f32)
    put("fnb", np.tile(W["fnb"][None, :], (128, 1)), f32)
    put("Wout", W["Wout"], f32)
    put("bout", np.tile(W["bout"][None, :], (128, 1)), f32)
    put("Wg1", W["Wg1"], f32)
    put("bg1_row", W["bg1"][None, :], f32)
    put("Wg2", W["Wg2"], f32)
    put("bg2_row", W["bg2"][None, :], f32)
    put("WplT", W["Wpl"].T, f32)
    # fold bq2 into the lin bias (preds = lin + quad, both biased)
    put("bplq_row", (W["bpl"] + W["bq2"])[None, :], f32)
    K4, F_DIM = cfg["K4"], cfg["F_DIM"]
    FH = F_DIM // 2
    put("Wq1f", W["Wq1"].transpose(1, 0, 2).reshape(F_DIM, K4 * FH), f32)
    put("bq1_row", W["bq1"].reshape(1, K4 * FH), f32)
    put("Wq2bc", np.tile(W["Wq2"].reshape(1, K4 * FH), (128, 1)), f32)

    in_maps = []
    for c in range(NC):
        m = dict(shared)
        m["src_idxT"] = np.ascontiguousarray(src_idxT[c])
        m["dst_relT"] = np.ascontiguousarray(dst_relT[c])
        m["ea1T"] = np.ascontiguousarray(ea1T[c].astype(npdt))
        m["ST_host"] = np.ascontiguousarray(ST_host[c])
        m["batchT"] = np.ascontiguousarray(batchT[c])
        m["xT1_shard"] = np.ascontiguousarray(xT1_shard[c].astype(npdt))
        in_maps.append(m)
    return in_maps, B_T


# ----------------------------------------------------------------------------
# kernel build
# ----------------------------------------------------------------------------

def _build(cfg, B_T):
    import concourse.bacc as bacc
    import concourse.bass as bass
    import concourse.mybir as mybir
    from concourse.tile import TileContext

    NC, PC, NB = cfg["NC"], cfg["PC"], cfg["NB"]
    H, L, G, GT = cfg["H"], cfg["L"], cfg["G"], cfg["GT"]
    N_PAD, EPS = cfg["N_PAD"], cfg["EPS"]
    K4, F_DIM = cfg["K4"], cfg["F_DIM"]
    FH = F_DIM // 2
    T = NB * B_T
    DT = mybir.dt.float32 if cfg["DT"] == "float32" else mybir.dt.bfloat16
    F32 = mybir.dt.float32
    AF = mybir.ActivationFunctionType
    OP = mybir.AluOpType
    ds, ts = bass.ds, bass.ts

    nc = bacc.Bacc("TRN2")

    # ---- I/O ----
    inp = {}

    def di(name, shape, dt=DT):
        inp[name] = nc.dram_tensor(name, shape, dt, kind="ExternalInput")
        return inp[name]

    src_idxT = di("src_idxT", [128, T], mybir.dt.int32)
    dst_relT = di("dst_relT", [128, T], F32)
    ea1T = di("ea1T", [5, T * 128])
    ST_host = di("ST_host", [128, T * 128])
    batchT = di("batchT", [128, NB], F32)
    xT1_shard = di("xT1_shard", [8, PC])
    Win1 = di("Win1", [8, H])
    Wdst = [di(f"Wdst{l}", [H, 2 * H]) for l in range(L)]
    Wsrc = [di(f"Wsrc{l}", [H, 2 * H]) for l in range(L)]
    Wfse1 = [di(f"Wfse1{l}", [5, 2 * H]) for l in range(L)]
    lng = [di(f"lng{l}", [128, H], F32) for l in range(L)]
    lnb = [di(f"lnb{l}", [128, H], F32) for l in range(L)]
    identity_dt = di("identity_dt", [128, 128])
    identity_f32 = di("identity_f32", [128, 128], F32)
    iota128 = di("iota128", [128, 128], F32)
    iota256 = di("iota256", [128, G], F32)
    ones_dt = di("ones_dt", [128, 1])
    eps_col = di("eps_col", [128, 1], F32)
    one_col = di("one_col", [128, 1], F32)
    ones_row = di("ones_row", [1, 128], F32)
    tdaT1 = di("tdaT1", [cfg["TDA_DIM"] + 1, G], F32)
    Wt11 = di("Wt11", [cfg["TDA_DIM"] + 1, 2 * cfg["TDA_PROJ"]], F32)
    tg = di("tg", [128, 2 * cfg["TDA_PROJ"]], F32)
    tb = di("tb", [128, 2 * cfg["TDA_PROJ"]], F32)
    Wt2 = di("Wt2", [2 * cfg["TDA_PROJ"], cfg["TDA_PROJ"]], F32)
    bt2 = di("bt2", [128, cfg["TDA_PROJ"]], F32)
    fng = di("fng", [128, F_DIM], F32)
    fnb = di("fnb", [128, F_DIM], F32)
    Wout = di("Wout", [H, H], F32)
    bout = di("bout", [128, H], F32)
    Wg1 = di("Wg1", [F_DIM, K4 * 4], F32)
    bg1_row = di("bg1_row", [1, K4 * 4], F32)
    Wg2 = di("Wg2", [K4 * 4, K4], F32)
    bg2_row = di("bg2_row", [1, K4], F32)
    WplT = di("WplT", [F_DIM, K4], F32)
    bplq_row = di("bplq_row", [1, K4], F32)
    Wq1f = di("Wq1f", [F_DIM, K4 * FH], F32)
    bq1_row = di("bq1_row", [1, K4 * FH], F32)
    Wq2bc = di("Wq2bc", [128, K4 * FH], F32)

    y_out = nc.dram_tensor("y", [G], F32, kind="ExternalOutput")
    z_out = nc.dram_tensor("z", [G, F_DIM], F32, kind="ExternalOutput")
    DBG = cfg.get("DEBUG", False)
    if DBG:
        dbg_h = [nc.dram_tensor(f"dbg_h{i}", [128, PC], F32,
                                kind="ExternalOutput") for i in range(L + 1)]
        dbg_agg = [nc.dram_tensor(f"dbg_agg{i}", [128, PC], F32,
                                  kind="ExternalOutput") for i in range(L)]
        dbg_pool = nc.dram_tensor("dbg_pool", [128, G + 2], F32,
                                  kind="ExternalOutput")

    # ---- internal DRAM ----
    h_shardT_d = nc.dram_tensor("h_shardT_d", [128, PC], DT)
    h_fullT = nc.dram_tensor("h_fullT", [NC * 128, PC], DT, addr_space="Shared")
    B_table = nc.dram_tensor("B_table", [N_PAD, 2 * H], DT)
    pool_in = nc.dram_tensor("pool_in", [128, G + 2], F32)
    pool_out = nc.dram_tensor("pool_out", [128, G + 2], F32, addr_space="Shared")

    groups = [list(range(NC))]

    from contextlib import ExitStack
    with nc.allow_low_precision(reason="bf16 transposes/copies by design"), \
         TileContext(nc) as tc:
        with (
            tc.tile_pool(name="const", bufs=1) as cp,
            tc.tile_pool(name="res", bufs=1) as rp,
            tc.tile_pool(name="node", bufs=2) as np_,
            tc.tile_pool(name="ps_t", bufs=2, space="PSUM") as ps_t,
            tc.tile_pool(name="ps_f", bufs=2, space="PSUM") as ps_f,
            tc.tile_pool(name="ps_a", bufs=2, space="PSUM") as ps_a,
            tc.tile_pool(name="ps_b", bufs=1, space="PSUM") as ps_b,
        ):
            _layer_stack = ExitStack()
            bp = _layer_stack.enter_context(tc.tile_pool(name="blk", bufs=2))
            tp = _layer_stack.enter_context(tc.tile_pool(name="tile", bufs=4))
            bbp = _layer_stack.enter_context(tc.tile_pool(name="bb", bufs=2))
            # ---- load constants ----
            def load_const(t, dt=DT):
                tile = cp.tile(list(t.shape), dt, tag=t.name)
                nc.sync.dma_start(out=tile[:], in_=t[:])
                return tile

            c_id = load_const(identity_dt)
            c_idf = load_const(identity_f32, F32)
            c_iota = load_const(iota128, F32)
            c_iota256 = load_const(iota256, F32)
            c_ones = load_const(ones_dt)
            c_eps = load_const(eps_col, F32)
            c_one = load_const(one_col, F32)
            c_ones_row = load_const(ones_row, F32)
            c_Win1 = load_const(Win1)
            c_Wdst = [load_const(Wdst[l]) for l in range(L)]
            c_Wsrc = [load_const(Wsrc[l]) for l in range(L)]
            c_Wfse1 = [load_const(Wfse1[l]) for l in range(L)]
            c_lng = [load_const(lng[l], F32) for l in range(L)]
            c_lnb = [load_const(lnb[l], F32) for l in range(L)]
            c_batchT = load_const(batchT, F32)

            hsh = rp.tile([128, PC], DT)
            hshT = rp.tile([128, PC], DT)
            agg = rp.tile([128, PC], F32)

            # ---------------- layer-norm helper (in/out [128, W] f32) -------
            def layer_norm(dst_ap, src_ap, w, g_tile, b_tile, pool):
                nm = pool.tile([128, 1], F32, tag="ln_nm")
                nm2 = pool.tile([128, 1], F32, tag="ln_nm2")
                xc = pool.tile([128, w], F32, tag="ln_xc")
                sq = pool.tile([128, w], F32, tag="ln_sq")
                vs = pool.tile([128, 1], F32, tag="ln_vs")
                rs = pool.tile([128, 1], F32, tag="ln_rs")
                nc.vector.tensor_reduce(out=nm[:], in_=src_ap,
                                        axis=mybir.AxisListType.X, op=OP.add,
                                        negate=True)
                nc.scalar.activation(nm2[:], nm[:], AF.Copy, scale=1.0 / w)
                nc.vector.tensor_scalar_add(out=xc[:], in0=src_ap, scalar1=nm2[:])
                nc.scalar.activation(sq[:], xc[:], AF.Square, accum_out=vs[:])
                sv = pool.tile([128, 1], F32, tag="ln_sv")
                nc.scalar.activation(sv[:], vs[:], AF.Ln, scale=1.0 / w,
                                     bias=c_eps[:])
                nc.scalar.activation(rs[:], sv[:], AF.Exp, scale=-0.5)
                tmp = pool.tile([128, w], F32, tag="ln_tmp")
                nc.scalar.activation(tmp[:], xc[:], AF.Copy, scale=rs[:])
                tmp2 = pool.tile([128, w], F32, tag="ln_tmp2")
                nc.vector.tensor_tensor(out=tmp2[:], in0=tmp[:], in1=g_tile,
                                        op=OP.mult)
                nc.vector.tensor_tensor(out=dst_ap, in0=tmp2[:], in1=b_tile,
                                        op=OP.add)

            def silu_op(out_ap, in_ap, w, pool, tagp):
                u = pool.tile([128, w], F32, tag="sl_u", name="sl_u")
                nc.scalar.activation(u[:], in_ap, AF.Exp, scale=-1.0)
                t = pool.tile([128, w], F32, tag="sl_t", name="sl_t")
                nc.vector.tensor_scalar_add(out=t[:], in0=u[:], scalar1=1.0)
                r = pool.tile([128, w], F32, tag="sl_r", name="sl_r")
                nc.vector.reciprocal_approx_fast(out=r[:], in_=t[:])
                nc.vector.tensor_tensor(out=out_ap, in0=in_ap, in1=r[:],
                                        op=OP.mult)

            # ---------------- phase 0: h0 ----------------
            def h0_body(tb):
                xt = np_.tile([8, 128], DT, tag="h0_x")
                nc.sync.dma_start(out=xt[:], in_=xT1_shard[:, ts(tb, 128)])
                pm = ps_t.tile([128, 128], F32, tag="pt128")
                nc.tensor.matmul(pm[:], xt[:], c_Win1[:], start=True, stop=True)
                h0t = np_.tile([128, 128], DT, tag="h0_t")
                silu_op(h0t[:], pm[:], 128, np_, "h0s")
                nc.vector.tensor_copy(out=hsh[:, ts(tb, 128)], in_=h0t[:])
                pt = ps_t.tile([128, 128], DT, tag="pt128")
                nc.tensor.transpose(pt[:], h0t[:], c_id[:])
                nc.scalar.copy(out=hshT[:, ts(tb, 128)], in_=pt[:])

            tc.For_i_unrolled(0, NB, 1, h0_body, max_unroll=min(8, NB))
            if DBG:
                nc.sync.dma_start(out=dbg_h[0][:], in_=hsh[:])
            nc.sync.dma_start(out=h_shardT_d[:], in_=hshT[:])
            nc.gpsimd.collective_compute(
                "AllGather", OP.bypass, ins=[h_shardT_d[:]], outs=[h_fullT[:]],
                replica_groups=groups)

            # ---------------- layers ----------------
            for l in range(L):
                # -- build B table from h_fullT --
                def bb_body(g, s):
                    hT = bbp.tile([128, 384], DT, tag="bb_h")
                    nc.sync.dma_start(
                        out=hT[:],
                        in_=h_fullT[s * 128:(s + 1) * 128, ds(g * 384, 384)])
                    pm = ps_b.tile([128, 768], F32, tag="pb768")
                    for i in range(3):
                        nc.tensor.matmul(pm[:, i * 256:(i + 1) * 256],
                                         hT[:, i * 128:(i + 1) * 128],
                                         c_Wsrc[l][:], start=True, stop=True)
                    bt = bbp.tile([128, 768], DT, tag="bb_bt")
                    nc.scalar.copy(out=bt[:], in_=pm[:])
                    out_ap = B_table[ds(s * PC + g * 384, 384), :].rearrange(
                        "(i p) d -> p i d", p=128)
                    nc.sync.dma_start(out=out_ap,
                                      in_=bt[:].rearrange("p (i d) -> p i d", i=3))

                for s in range(NC):
                    tc.For_i_unrolled(0, NB // 3, 1,
                                      lambda g, s=s: bb_body(g, s),
                                      max_unroll=min(11, NB // 3))

                # -- edge phase --
                def edge_body(b):
                    idx_sb = bp.tile([128, B_T], mybir.dt.int32, tag="e_idx")
                    nc.sync.dma_start(out=idx_sb[:], in_=src_idxT[:, ts(b, B_T)])
                    rel_sb = bp.tile([128, B_T], F32, tag="e_rel")
                    nc.sync.dma_start(out=rel_sb[:], in_=dst_relT[:, ts(b, B_T)])
                    ea_sb = bp.tile([5, B_T * 128], DT, tag="e_ea")
                    nc.sync.dma_start(out=ea_sb[:], in_=ea1T[:, ts(b, B_T * 128)])
                    blkT = bp.tile([128, 128], DT, tag="e_hT")
                    nc.vector.tensor_copy(out=blkT[:], in_=hshT[:, ts(b, 128)])
                    pa = ps_b.tile([128, 256], F32, tag="pb768")
                    nc.tensor.matmul(pa[:], blkT[:], c_Wdst[l][:],
                                     start=True, stop=True)
                    A_sb = bp.tile([128, 256], DT, tag="e_A")
                    nc.scalar.copy(out=A_sb[:], in_=pa[:])
                    S_cat = bp.tile([128, B_T * 128], DT, tag="e_S")
                    nc.vector.tensor_tensor(
                        out=S_cat[:].rearrange("p (t w) -> p t w", t=B_T),
                        in0=rel_sb[:].unsqueeze(2).to_broadcast([128, B_T, 128]),
                        in1=c_iota[:].unsqueeze(1).to_broadcast([128, B_T, 128]),
                        op=OP.is_equal)
                    ST_cat = bp.tile([128, B_T * 128], DT, tag="e_STc")
                    nc.sync.dma_start(out=ST_cat[:],
                                      in_=ST_host[:, ts(b, B_T * 128)])
                    pagg = ps_a.tile([128, 128], F32, tag="pa128")
                    for i in range(B_T):
                        pf = ps_f.tile([128, 256], F32, tag="pf")
                        nc.tensor.matmul(pf[:], ST_cat[:, i * 128:(i + 1) * 128],
                                         A_sb[:], start=True,
                                         stop=False)
                        nc.tensor.matmul(pf[:], ea_sb[:, i * 128:(i + 1) * 128],
                                         c_Wfse1[l][:], start=False, stop=True)
                        Gt = tp.tile([128, 256], DT, tag="e_G")
                        nc.gpsimd.indirect_dma_start(
                            out=Gt[:], out_offset=None, in_=B_table[:],
                            in_offset=bass.IndirectOffsetOnAxis(
                                ap=idx_sb[:, i:i + 1], axis=0))
                        Ft = tp.tile([128, 256], DT, tag="e_F")
                        nc.vector.tensor_tensor(out=Ft[:], in0=Gt[:], in1=pf[:],
                                                op=OP.add)
                        # tables carry -f || s, so exp gives e^{-f} || e^{s}
                        uv = tp.tile([128, 256], DT, tag="e_uv")
                        nc.scalar.activation(uv[:], Ft[:], AF.Exp)
                        t1 = tp.tile([128, 128], F32, tag="e_t1")
                        nc.vector.tensor_scalar_add(out=t1[:], in0=uv[:, 0:128],
                                                    scalar1=1.0)
                        rc = tp.tile([128, 128], F32, tag="e_rc")
                        nc.vector.reciprocal_approx_fast(out=rc[:], in_=t1[:])
                        lv = tp.tile([128, 128], DT, tag="e_lv")
                        nc.scalar.activation(lv[:], uv[:, 128:256], AF.Ln,
                                             bias=c_one[:])
                        msg = tp.tile([128, 128], DT, tag="e_msg")
                        nc.vector.tensor_tensor(out=msg[:], in0=rc[:], in1=lv[:],
                                                op=OP.mult)
                        nc.tensor.matmul(pagg[:],
                                         S_cat[:, i * 128:(i + 1) * 128],
                                         msg[:], start=(i == 0),
                                         stop=(i == B_T - 1))
                    nc.scalar.copy(out=agg[:, ts(b, 128)], in_=pagg[:])

                tc.For_i_unrolled(0, NB, 1, edge_body, max_unroll=min(2, NB))
                if DBG:
                    nc.sync.dma_start(out=dbg_agg[l][:], in_=agg[:])

                # -- node update --
                def node_body(tb):
                    sl = ts(tb, 128)
                    conv = np_.tile([128, 128], F32, tag="n_conv")
                    nc.vector.tensor_tensor(out=conv[:], in0=agg[:, sl],
                                            in1=hsh[:, sl], op=OP.add)
                    t2a = np_.tile([128, 128], F32, tag="n_t2a")
                    silu_op(t2a[:], conv[:], 128, np_, "ns")
                    t2 = np_.tile([128, 128], F32, tag="n_t2")
                    nc.vector.tensor_tensor(out=t2[:], in0=t2a[:], in1=hsh[:, sl],
                                            op=OP.add)
                    nh = np_.tile([128, 128], DT, tag="n_nh")
                    layer_norm(nh[:], t2[:], 128, c_lng[l][:], c_lnb[l][:],
                               np_)
                    nc.vector.tensor_copy(out=hsh[:, sl], in_=nh[:])
                    pt = ps_t.tile([128, 128], DT, tag="pt128")
                    nc.tensor.transpose(pt[:], nh[:], c_id[:])
                    nc.scalar.copy(out=hshT[:, sl], in_=pt[:])

                tc.For_i_unrolled(0, NB, 1, node_body, max_unroll=min(4, NB))
                if DBG:
                    nc.sync.dma_start(out=dbg_h[l + 1][:], in_=hsh[:])

                if l < L - 1:
                    nc.sync.dma_start(out=h_shardT_d[:], in_=hshT[:])
                    nc.gpsimd.collective_compute(
                        "AllGather", OP.bypass, ins=[h_shardT_d[:]],
                        outs=[h_fullT[:]], replica_groups=groups)

            # ---------------- pooling ----------------
            _layer_stack.close()
            _head_stack = ExitStack()
            hp2 = _head_stack.enter_context(tc.tile_pool(name="head", bufs=2))
            p_pool = ps_b.tile([128, G], F32, tag="pb768")
            p_cnts = [ps_a.tile([128, 1], F32, tag="pa128", name=f"p_cnt{_i}")
                      for _i in range(GT)]

            def pool_body(tb, first, last):
                sp_t = hp2.tile([128, G], DT, tag="p_S", name="sp_t")
                nc.vector.tensor_tensor(
                    out=sp_t[:],
                    in0=c_batchT[:, ds(tb, 1)].to_broadcast([128, G]),
                    in1=c_iota256[:], op=OP.is_equal)
                hcp = hp2.tile([128, 128], DT, tag="p_h", name="hcp")
                nc.vector.tensor_copy(out=hcp[:], in_=hsh[:, ts(tb, 128)])
                nc.tensor.matmul(p_pool[:], hcp[:], sp_t[:],
                                 start=first, stop=last)
                for hh in range(GT):
                    nc.tensor.matmul(p_cnts[hh][:],
                                     sp_t[:, hh * 128:(hh + 1) * 128],
                                     c_ones[:], start=first, stop=last)

            pool_body(0, True, False)
            if NB > 2:
                tc.For_i_unrolled(1, NB - 1, 1,
                                  lambda tb: pool_body(tb, False, False),
                                  max_unroll=min(8, NB - 2))
            pool_body(NB - 1, False, True)

            pool_sb = hp2.tile([128, G + 2], F32, tag="p_sb", name="pool_sb")
            nc.scalar.copy(out=pool_sb[:, 0:G], in_=p_pool[:])
            for hh in range(GT):
                nc.scalar.copy(out=pool_sb[:, G + hh:G + hh + 1], in_=p_cnts[hh][:])
            nc.sync.dma_start(out=pool_in[:], in_=pool_sb[:])
            nc.gpsimd.collective_compute(
                "AllReduce", OP.add, ins=[pool_in[:]], outs=[pool_out[:]],
                replica_groups=groups)
            pl = hp2.tile([128, G + 2], F32, tag="p_pl", name="pl")
            nc.sync.dma_start(out=pl[:], in_=pool_out[:])
            if DBG:
                nc.sync.dma_start(out=dbg_pool[:], in_=pool_out[:])

            # ---------------- head ----------------
            hp = hp2  # pool for head tiles
            c_tdaT1 = load_const(tdaT1, F32)
            c_Wt11 = load_const(Wt11, F32)
            c_tg = load_const(tg, F32)
            c_tb = load_const(tb, F32)
            c_Wt2 = load_const(Wt2, F32)
            c_bt2 = load_const(bt2, F32)
            c_fng = load_const(fng, F32)
            c_fnb = load_const(fnb, F32)
            c_Wout = load_const(Wout, F32)
            c_bout = load_const(bout, F32)
            def load_const_pair(t):
                a = cp.tile([128, t.shape[1]], F32, tag=t.name + "_a")
                nc.sync.dma_start(out=a[:], in_=t[0:128, :])
                b = cp.tile([t.shape[0] - 128, t.shape[1]], F32, tag=t.name + "_b")
                nc.sync.dma_start(out=b[:], in_=t[128:, :])
                return a, b

            c_Wg1a, c_Wg1b = load_const_pair(Wg1)
            c_bg1r = load_const(bg1_row, F32)
            c_Wg2 = load_const(Wg2, F32)
            c_bg2r = load_const(bg2_row, F32)
            c_WplTa, c_WplTb = load_const_pair(WplT)
            c_bplqr = load_const(bplq_row, F32)
            c_Wq1fa, c_Wq1fb = load_const_pair(Wq1f)
            c_bq1r = load_const(bq1_row, F32)
            c_Wq2bc = load_const(Wq2bc, F32)

            # counts reciprocal per half: pl[:, G+hh]
            rcp = []
            for hh in range(GT):
                cc = hp.tile([128, 1], F32, tag=f"hd_c{hh}")
                nc.vector.tensor_scalar_max(out=cc[:], in0=pl[:, G + hh:G + hh + 1],
                                            scalar1=1.0)
                rr = hp.tile([128, 1], F32, tag=f"hd_r{hh}")
                nc.vector.reciprocal(rr[:], cc[:])
                rcp.append(rr)

            zT0 = hp.tile([128, G], F32, tag="hd_zT0")
            zT1 = hp.tile([64, G], F32, tag="hd_zT1")
            for hh in range(GT):
                gsl = slice(hh * 128, (hh + 1) * 128)
                # z_gnn = (pooledT.T @ Wout) * (1/cnt) + bout
                pz = ps_f.tile([128, 128], F32, tag="pf")
                nc.tensor.matmul(pz[:], pl[:, gsl], c_Wout[:], start=True,
                                 stop=True)
                zg = hp.tile([128, F_DIM], F32, tag="hd_z")
                nc.scalar.activation(zg[:, 0:H], pz[:], AF.Copy, scale=rcp[hh][:])
                nc.vector.tensor_tensor(out=zg[:, 0:H], in0=zg[:, 0:H],
                                        in1=c_bout[:], op=OP.add)
                # tda projector
                pt1 = ps_f.tile([128, 2 * cfg["TDA_PROJ"]], F32, tag="pf")
                nc.tensor.matmul(pt1[:], c_tdaT1[:, gsl], c_Wt11[:], start=True,
                                 stop=True)
                t1 = hp.tile([128, 2 * cfg["TDA_PROJ"]], F32, tag="hd_t1")
                silu_op(t1[:], pt1[:], 2 * cfg["TDA_PROJ"], hp, "hs1")
                t1n = hp.tile([128, 2 * cfg["TDA_PROJ"]], F32, tag="hd_t1n")
                layer_norm(t1n[:], t1[:], 2 * cfg["TDA_PROJ"], c_tg[:], c_tb[:],
                           hp)
                ptt = ps_t.tile([128, 128], F32, tag="pt128")
                nc.tensor.transpose(ptt[:], t1n[:], c_idf[:])
                t1T = hp.tile([128, 128], F32, tag="hd_t1T")
                nc.scalar.copy(out=t1T[:], in_=ptt[:])
                pzt = ps_f.tile([128, cfg["TDA_PROJ"]], F32, tag="pf")
                nc.tensor.matmul(pzt[:], t1T[:], c_Wt2[:], start=True, stop=True)
                nc.vector.tensor_tensor(out=zg[:, H:F_DIM], in0=pzt[:],
                                        in1=c_bt2[:], op=OP.add)
                # fusion layernorm
                zf = hp.tile([128, F_DIM], F32, tag="hd_zf")
                layer_norm(zf[:], zg[:], F_DIM, c_fng[:], c_fnb[:], hp)
                nc.sync.dma_start(out=z_out[gsl, :], in_=zf[:])
                # transposes into zT0/zT1
                pz0 = ps_t.tile([128, 128], F32, tag="pt128")
                nc.tensor.transpose(pz0[:], zf[:, 0:128], c_idf[:])
                nc.scalar.copy(out=zT0[:, gsl], in_=pz0[:])
                pz1 = ps_t.tile([64, 128], F32, tag="pt128")
                nc.tensor.transpose(pz1[:], zf[:, 128:192], c_idf[:])
                nc.scalar.copy(out=zT1[:, gsl], in_=pz1[:])

            for hh in range(GT):
                gsl = slice(hh * 128, (hh + 1) * 128)
                # gates
                pg1 = ps_f.tile([128, K4 * 4], F32, tag="pf")
                nc.tensor.matmul(pg1[:], zT0[:, gsl], c_Wg1a[:],
                                 start=True, stop=False)
                nc.tensor.matmul(pg1[:], zT1[:, gsl], c_Wg1b[:],
                                 start=False, stop=False)
                nc.tensor.matmul(pg1[:], c_ones_row[:], c_bg1r[:],
                                 start=False, stop=True)
                g1 = hp.tile([128, K4 * 4], F32, tag="hd_g1")
                silu_op(g1[:], pg1[:], K4 * 4, hp, "hs2")
                pg1t = ps_t.tile([16, 128], F32, tag="pt128")
                nc.tensor.transpose(pg1t[:], g1[:], c_idf[:])
                g1T = hp.tile([16, 128], F32, tag="hd_g1T")
                nc.scalar.copy(out=g1T[:], in_=pg1t[:])
                pg2 = ps_f.tile([128, K4], F32, tag="pf")
                nc.tensor.matmul(pg2[:], g1T[:], c_Wg2[:], start=True, stop=False)
                nc.tensor.matmul(pg2[:], c_ones_row[:], c_bg2r[:],
                                 start=False, stop=True)
                ex = hp.tile([128, K4], F32, tag="hd_ex")
                se = hp.tile([128, 1], F32, tag="hd_se")
                nc.scalar.activation(ex[:], pg2[:], AF.Exp, accum_out=se[:])
                rse = hp.tile([128, 1], F32, tag="hd_rse")
                nc.vector.reciprocal(rse[:], se[:])
                gates = hp.tile([128, K4], F32, tag="hd_gates")
                nc.scalar.activation(gates[:], ex[:], AF.Copy, scale=rse[:])
                # lin (+ folded bq2)
                plin = ps_f.tile([128, K4], F32, tag="pf")
                nc.tensor.matmul(plin[:], zT0[:, gsl], c_WplTa[:],
                                 start=True, stop=False)
                nc.tensor.matmul(plin[:], zT1[:, gsl], c_WplTb[:],
                                 start=False, stop=False)
                nc.tensor.matmul(plin[:], c_ones_row[:], c_bplqr[:],
                                 start=False, stop=True)
                # quad
                pq = ps_b.tile([128, K4 * FH], F32, tag="pb768")
                nc.tensor.matmul(pq[:], zT0[:, gsl], c_Wq1fa[:],
                                 start=True, stop=False)
                nc.tensor.matmul(pq[:], zT1[:, gsl], c_Wq1fb[:],
                                 start=False, stop=False)
                nc.tensor.matmul(pq[:], c_ones_row[:], c_bq1r[:],
                                 start=False, stop=True)
                qs = hp.tile([128, K4 * FH], F32, tag="hd_qs")
                silu_op(qs[:], pq[:], K4 * FH, hp, "hs3")
                qw = hp.tile([128, K4 * FH], F32, tag="hd_qw")
                nc.vector.tensor_tensor(out=qw[:], in0=qs[:], in1=c_Wq2bc[:],
                                        op=OP.mult)
                quad = hp.tile([128, K4], F32, tag="hd_quad")
                nc.vector.tensor_reduce(
                    out=quad[:],
                    in_=qw[:].rearrange("p (k f) -> p k f", k=K4),
                    axis=mybir.AxisListType.X, op=OP.add)
                preds = hp.tile([128, K4], F32, tag="hd_preds")
                nc.vector.tensor_tensor(out=preds[:], in0=quad[:], in1=plin[:],
                                        op=OP.add)
                gp_t = hp.tile([128, K4], F32, tag="hd_gp")
                nc.vector.tensor_tensor(out=gp_t[:], in0=gates[:], in1=preds[:],
                                        op=OP.mult)
                yv = hp.tile([128, 1], F32, tag="hd_y")
                nc.vector.tensor_reduce(out=yv[:], in_=gp_t[:],
                                        axis=mybir.AxisListType.X, op=OP.add)
                nc.sync.dma_start(out=y_out[gsl].unsqueeze(1), in_=yv[:])
            _head_stack.close()

    nc.compile()
    return nc


# ----------------------------------------------------------------------------
# entry point
# ----------------------------------------------------------------------------

def run(cfg, inputs, trace=False):
    from concourse.bass_utils import run_bass_kernel_spmd

    cfg = _derived(cfg)
    in_maps, B_T = _prep(cfg, inputs)
    nc = _build(cfg, B_T)
    res = run_bass_kernel_spmd(nc, in_maps, list(range(cfg["NC"])), trace=trace)
    y = res.results[0]["y"][:cfg["G"]]
    z = res.results[0]["z"][:cfg["G"]]
    return (y, z), res


def kernel(**inputs):
    (y, z), _ = run(full_cfg(), inputs, trace=False)
    return y, z
